# revision 4
# baseline (speedup 1.0000x reference)
"""Cross-attention kernel for Trainium2, 8 NeuronCores — v2.

Problem: b=4, s=2048, d_model=1024, n_heads=16 (head_dim=64), fp32.
  out = softmax((q@Wq) (k@Wk)^T / sqrt(64) + mask) @ (v@Wv) @ Wo + bo

Sharding: core c handles batch c//2 and head-group c%2 (8 heads, 512
projection columns). Each core computes a partial output (s, 1024);
the host sums the two partials per batch and adds bo.

v2 changes over the baseline:
  * Paired QK^T matmuls via PE row tiling: heads 2b/2b+1 live on
    partitions 0:64 / 64:128, so their S^T matmuls occupy disjoint
    row groups and run concurrently on hardware (2x QK^T).
  * exp split across ScalarE (Exp activation, bf16 out) and VectorE
    (Schraudolph bf16 exp: one fused mult+add whose fp32 result's low
    16 bits ARE the bf16 exp — consumed through a stride-2 bf16 view).
  * Vh / AttnOut / Wo in bf16 (PE rate unchanged, SBUF/DMA halved).
  * V-projection bias folded into the matmul via a K=1 ones-row
    matmul (the bias varies along the free dim); Q/K biases applied
    by the per-partition-scalar add that also evacuates PSUM.
  * Phase C (out-proj) runs per i-chunk right after its attention
    output is staged in SBUF — no DRAM bounce, PE work overlaps the
    exp-bound attention phase.
  * Division by the softmax denominator: fast PSUM evacuation on
    ScalarE, then reciprocal + DRAM-bounce partition broadcast
    SBUF-side, off the AV accumulation critical path.
"""

import numpy as np

import concourse.bass as bass
import concourse.tile as tile
from concourse import mybir
from concourse.bass_utils import run_bass_kernel_spmd

P = 128
S = 2048          # sequence length
DIN = 1024        # model dim
C = 512           # projection columns per core (8 heads * 64)
NHC = 8           # heads per core
HD = 64           # head dim
HW1 = HD + 1      # 65: head cols incl. ones column
VW = NHC * HW1    # 520: head-interleaved V width incl. ones columns
NIC = S // 512    # 4 i-chunks
NJT = S // P      # 16 j-tiles
F32 = mybir.dt.float32
F32R = mybir.dt.float32r
BF16 = mybir.dt.bfloat16

# Schraudolph bf16 exp of (x/8): low 16 bits of fp32(x*A2 + B2).
A2 = 16.0 / np.log(2.0)              # 0.125 * 128/ln2
B2 = 16256.0 - 5.25 + 12582912.0     # bf16 bias+corr + 1.5*2^23 magic
# j-tiles handled by the DVE exp (rest on ScalarE): 5 of 16
DVE_JT = {0, 3, 7, 10, 13}


def _build_kernel():
    nc = bass.Bass("TRN2", target_bir_lowering=False, debug=False)

    qT = nc.dram_tensor("qT", [DIN, S], F32R, kind="ExternalInput").ap()
    kT = nc.dram_tensor("kT", [DIN, S], F32R, kind="ExternalInput").ap()
    vT = nc.dram_tensor("vT", [DIN, S], F32R, kind="ExternalInput").ap()
    wq = nc.dram_tensor("wq", [DIN, C], F32R, kind="ExternalInput").ap()
    wk = nc.dram_tensor("wk", [DIN, C], F32R, kind="ExternalInput").ap()
    wv = nc.dram_tensor("wv", [DIN, VW], F32R, kind="ExternalInput").ap()
    wo = nc.dram_tensor("wo", [C, DIN], BF16, kind="ExternalInput").ap()
    bq = nc.dram_tensor("bq", [C], F32, kind="ExternalInput").ap()
    bk = nc.dram_tensor("bk", [C], F32, kind="ExternalInput").ap()
    bva = nc.dram_tensor("bva", [1, VW], F32R, kind="ExternalInput").ap()
    onesd = nc.dram_tensor("onesd", [1, 512], F32R, kind="ExternalInput").ap()
    mm = nc.dram_tensor("mm", [S], F32, kind="ExternalInput").ap()
    y = nc.dram_tensor("y", [S, DIN], F32, kind="ExternalOutput").ap()

    rcp_dram = nc.dram_tensor("rcp_st", [32, 512], F32).ap()  # 1/denom bounce

    with tile.TileContext(nc) as tc:
        _body(tc, y, rcp_dram, qT, kT, vT, wq, wk, wv, wo, bq, bk, bva, mm,
              onesd)
    return nc


def _strided_bf16_view(ap):
    """bf16 view of an fp32 AP taking the low 16 bits of each element."""
    b = ap.bitcast(BF16)
    dims = list(b.ap)
    # innermost dim: [step, count] in bf16 elements; take every other one
    assert dims[-1][0] == 1
    dims[-1] = [2, dims[-1][1] // 2]
    return bass.AP(tensor=b.tensor, offset=b.offset, ap=dims)


def _body(tc, y, rcp_dram, qT, kT, vT, wq, wk, wv, wo, bq, bk, bva, mm,
          onesd, dumps=None):
    nc = tc.nc

    with tc.tile_pool(name="persist", bufs=1) as pp:
        qhT = pp.tile([P, NIC, S], F32R)    # [dout%128, dout//128, i]
        khT = pp.tile([P, NIC, S], F32R)
        vh = pp.tile([P, NJT, VW], BF16)    # [j%128, j//128, 8*(64+1)]
        ao = pp.tile([P, NIC, S], BF16)     # AttnOut^T [feat%128, feat//128, i]
        wo_sb = pp.tile([P, NIC, DIN], BF16)
        ones = pp.tile([1, 512], F32R)
        nc.sync.dma_start(out=ones, in_=onesd)

        # ---------------- Phase A: projections ----------------
        with (
            tc.tile_pool(name="wpool", bufs=1) as wpool,
            tc.tile_pool(name="xin", bufs=2) as xin,
        ):
            # K weights first: the first matmul can start after 256KB
            wk_kt = [wpool.tile([P, C], F32R, tag=f"wk{kt}", name=f"wk_kt{kt}") for kt in range(8)]
            for kt in range(8):
                nc.sync.dma_start(out=wk_kt[kt], in_=wk[kt * P:(kt + 1) * P, :])
            bk_sb = wpool.tile([P, 4], F32)
            nc.sync.dma_start(out=bk_sb, in_=bk.rearrange("(t p) -> p t", p=P))
            mm_sb = wpool.tile([P, NJT], F32)
            nc.sync.dma_start(out=mm_sb, in_=mm.rearrange("(t p) -> p t", p=P))

            def proj_qk(x_dram, w_kt, b_sb, dst):
                with tc.tile_pool(name="psqk", bufs=4, space="PSUM") as psp:
                    for ic in range(NIC):
                        xts = []
                        for kt in range(8):
                            xt = xin.tile([P, 512], F32R, tag=f"x{kt}")
                            nc.sync.dma_start(
                                out=xt,
                                in_=x_dram[kt * P:(kt + 1) * P,
                                           ic * 512:(ic + 1) * 512],
                            )
                            xts.append(xt)
                        for io in range(4):
                            ps = psp.tile([P, 512], F32, name="ps")
                            for kt in range(8):
                                nc.tensor.matmul(
                                    ps,
                                    (w_kt[kt][:, io * P:(io + 1) * P]),
                                    (xts[kt]),
                                    start=(kt == 0),
                                    stop=(kt == 7),
                                )
                            nc.vector.tensor_scalar_add(
                                out=dst[:, io, ic * 512:(ic + 1) * 512],
                                in0=ps,
                                scalar1=b_sb[:, io:io + 1],
                            )

            proj_qk(kT, wk_kt, bk_sb, khT)
            nc.sync.dma_start(out=wo_sb, in_=wo.rearrange("(t p) c -> p t c", p=P))

            wv_kt = [wpool.tile([P, VW], F32R, tag=f"wv{kt}", name=f"wv_kt{kt}") for kt in range(8)]
            for kt in range(8):
                nc.sync.dma_start(out=wv_kt[kt], in_=wv[kt * P:(kt + 1) * P, :])
            bva_sb = wpool.tile([1, VW], F32R)
            nc.sync.dma_start(out=bva_sb, in_=bva)

            # Vh: dst[j, c] = sum_k vT[k, j] * Wv[k, c] + bva[c]; * mask
            with tc.tile_pool(name="psv", bufs=3, space="PSUM") as psv:
                for jg in range(4):           # groups of 4 j-tiles
                    xts = []
                    for kt in range(8):
                        xt = xin.tile([P, 512], F32R, tag=f"x{kt}")
                        nc.sync.dma_start(
                            out=xt,
                            in_=vT[kt * P:(kt + 1) * P,
                                   jg * 512:(jg + 1) * 512],
                        )
                        xts.append(xt)
                    for ji in range(4):
                        jt = jg * 4 + ji
                        ps = psv.tile([P, VW], F32)
                        for kt in range(8):
                            nc.tensor.matmul(
                                ps[:, 0:512],
                                (xts[kt][:, ji * P:(ji + 1) * P]),
                                (wv_kt[kt][:, 0:512]),
                                start=(kt == 0),
                                stop=False,
                            )
                        nc.tensor.matmul(
                            ps[:, 0:512],
                            (ones[0:1, 0:P]),
                            (bva_sb[0:1, 0:512]),
                            start=False,
                            stop=True,
                        )
                        for kt in range(8):
                            nc.tensor.matmul(
                                ps[:, 512:VW],
                                (xts[kt][:, ji * P:(ji + 1) * P]),
                                (wv_kt[kt][:, 512:VW]),
                                start=(kt == 0),
                                stop=False,
                            )
                        nc.tensor.matmul(
                            ps[:, 512:VW],
                            (ones[0:1, 0:P]),
                            (bva_sb[0:1, 512:VW]),
                            start=False,
                            stop=True,
                        )
                        # mask rows & cast to bf16
                        nc.vector.tensor_scalar_mul(
                            out=vh[:, jt, :],
                            in0=ps,
                            scalar1=mm_sb[:, jt:jt + 1],
                        )

            wq_kt = [wpool.tile([P, C], F32R, tag=f"wq{kt}", name=f"wq_kt{kt}") for kt in range(8)]
            for kt in range(8):
                nc.sync.dma_start(out=wq_kt[kt], in_=wq[kt * P:(kt + 1) * P, :])
            bq_sb = wpool.tile([P, 4], F32)
            nc.sync.dma_start(out=bq_sb, in_=bq.rearrange("(t p) -> p t", p=P))
            proj_qk(qT, wq_kt, bq_sb, qhT)

        # ---------------- Phase B + C: attention & out-proj ----------------
        with (
            tc.tile_pool(name="st", bufs=3, space="PSUM") as stp,
            tc.tile_pool(name="ot", bufs=1, space="PSUM") as otp,
            tc.tile_pool(name="ex", bufs=6) as expool,
            tc.tile_pool(name="divp", bufs=3) as divp,
            tc.tile_pool(name="ysb", bufs=3) as ysb,
        ):
            def phase_c(ic):
                # out-projection for i-chunk ic: Y rows = AttnOut @ Wo
                for it in range(4):
                    i0 = (ic * 4 + it) * P
                    for ec in range(2):
                        ps = stp.tile([P, 512], F32, tag="st")
                        for ft in range(4):
                            nc.tensor.matmul(
                                ps,
                                (ao[:, ft, i0:i0 + P]),
                                (wo_sb[:, ft, ec * 512:(ec + 1) * 512]),
                                start=(ft == 0),
                                stop=(ft == 3),
                            )
                        yt = ysb.tile([P, 512], F32)
                        nc.vector.tensor_copy(out=yt, in_=ps)
                        nc.sync.dma_start(
                            out=y[i0:i0 + P, ec * 512:(ec + 1) * 512], in_=yt
                        )

            for ic in range(NIC):
                ics = slice(ic * 512, (ic + 1) * 512)
                for pr in range(4):           # head pair (block) index
                    if pr == 2 and ic > 0:
                        phase_c(ic - 1)   # deferred out-proj of previous chunk
                    otA = otp.tile([HW1, 512], F32, tag="otA")
                    otB = otp.tile([HW1, 512], F32, tag="otB")
                    # stage 1: all paired S^T matmuls + exp for the pair;
                    # stage 2: the AV accumulations. Decoupling keeps the PE
                    # from stalling on each chunk's exp.
                    exs = []
                    for jt in range(NJT):
                        st = stp.tile([P, 1024], F32, tag="st")
                        # paired S^T: rows 0:64 (head 2pr), 64:128 (head 2pr+1)
                        nc.tensor.matmul(
                            st[:, 0:512],
                            (khT[0:HD, pr, jt * P:(jt + 1) * P]),
                            (qhT[0:HD, pr, ics]),
                            start=True, stop=True,
                        )
                        nc.tensor.matmul(
                            st[:, 512:1024],
                            (khT[HD:P, pr, jt * P:(jt + 1) * P]),
                            (qhT[HD:P, pr, ics]),
                            start=True, stop=True,
                        )
                        if jt in DVE_JT:
                            ex32 = expool.tile([P, 1024], F32, tag="ex32",
                                               bufs=7)
                            nc.vector.tensor_scalar(
                                out=ex32, in0=st,
                                scalar1=float(A2), scalar2=float(B2),
                                op0=mybir.AluOpType.mult,
                                op1=mybir.AluOpType.add,
                            )
                            exs.append((_strided_bf16_view(ex32[:, 0:512]),
                                        _strided_bf16_view(ex32[:, 512:1024])))
                        else:
                            ex = expool.tile([P, 1024], BF16, tag="ex",
                                             bufs=11)
                            nc.scalar.activation(
                                out=ex, in_=st,
                                func=mybir.ActivationFunctionType.Exp,
                                scale=0.125,
                            )
                            exs.append((ex[:, 0:512], ex[:, 512:1024]))
                    for jt in range(NJT):
                        exA, exB = exs[jt]
                        nc.tensor.matmul(
                            otA,
                            (vh[:, jt, 2 * pr * HW1:(2 * pr + 1) * HW1]),
                            (exA),
                            start=(jt == 0), stop=(jt == NJT - 1),
                        )
                        nc.tensor.matmul(
                            otB,
                            (vh[:, jt, (2 * pr + 1) * HW1:(2 * pr + 2) * HW1]),
                            (exB),
                            start=(jt == 0), stop=(jt == NJT - 1),
                        )
                    # numerator / denominator for both heads of the pair:
                    # evacuate PSUM quickly (frees the ot bank for the next
                    # pair), then divide SBUF-side off the critical path
                    for u, ot in ((0, otA), (1, otB)):
                        onum = divp.tile([HW1, 512], F32, tag="onum")
                        nc.scalar.activation(
                            out=onum, in_=ot,
                            func=mybir.ActivationFunctionType.Copy,
                        )
                        rcp = divp.tile([1, 512], F32, tag="rcp")
                        nc.vector.reciprocal(out=rcp, in_=onum[HD:HW1, :])
                        # broadcast 1/denom over 64 partitions via a DRAM
                        # round trip (no partition-broadcast engine op in
                        # this walrus build)
                        slot = (ic * 8 + 2 * pr + u) % 32
                        nc.sync.dma_start(
                            out=rcp_dram[slot:slot + 1, :], in_=rcp
                        )
                        bc = divp.tile([HD, 512], F32, tag="bc")
                        row = rcp_dram[slot:slot + 1, :]
                        nc.sync.dma_start(
                            out=bc,
                            in_=bass.AP(tensor=row.tensor, offset=row.offset,
                                        ap=[[0, HD]] + list(row.ap)[1:]),
                        )
                        nc.vector.tensor_tensor(
                            out=ao[u * HD:(u + 1) * HD, pr, ics],
                            in0=onum[0:HD, :],
                            in1=bc,
                            op=mybir.AluOpType.mult,
                        )
            phase_c(NIC - 1)

        if dumps is not None:
            nc.sync.dma_start(out=dumps["dq"], in_=qhT)
            nc.sync.dma_start(out=dumps["dk"], in_=khT)
            nc.sync.dma_start(out=dumps["dv"], in_=vh)
            nc.sync.dma_start(out=dumps["dao"], in_=ao)


def _legalize_sync(bir, max_waits=1, max_updates=1):
    """Split sync lists so every instruction carries at most `max_waits`
    waits and `max_updates` updates; the walrus build in this container
    rejects instructions with more ("Too many sync wait commands").
    Extra waits go on EventSemaphore instructions inserted just before
    (same engine => same program order), extra updates just after."""
    n = [0]

    def ev(engine, debug, waits, updates):
        n[0] += 1
        return {
            "debug": debug,
            "engine": engine,
            "ins": [],
            "outs": [],
            "name": f"I-syncsplit-{n[0]}",
            "opcode": "EventSemaphore",
            "sync_info": {"on_wait": waits, "on_update": updates},
        }

    for fn in bir["functions"]:
        for bb in fn["blocks"]:
            out = []
            for ins in bb["instructions"]:
                si = ins.get("sync_info")
                eng = ins.get("engine")
                post = []
                if si and eng:
                    waits = si.get("on_wait") or []
                    updates = si.get("on_update") or []
                    dbg = ins.get("debug", 0)
                    while len(waits) > max_waits:
                        chunk, waits = waits[:max_waits], waits[max_waits:]
                        out.append(ev(eng, dbg, chunk, []))
                    while len(updates) > max_updates:
                        updates, chunk = updates[:-max_updates], updates[-max_updates:]
                        post.append(ev(eng, dbg, [], chunk))
                    si["on_wait"] = waits
                    si["on_update"] = updates
                out.append(ins)
                out.extend(reversed(post))
            bb["instructions"] = out


_NC_CACHE = {}


def _get_nc():
    if "nc" not in _NC_CACHE:
        import json as _json

        nc = _build_kernel()
        orig = nc.to_json_bytes

        def patched():
            bir = _json.loads(orig())
            _legalize_sync(bir)
            return _json.dumps(bir).encode()

        nc.to_json_bytes = patched
        _NC_CACHE["nc"] = nc
    return _NC_CACHE["nc"]


def make_in_maps(q, k, v, attention_mask, Wq, bq, Wk, bk, Wv, bv, Wo, bo):
    """Host-side sharding: returns the per-core input maps."""
    import ml_dtypes

    q = np.asarray(q, np.float32)
    k = np.asarray(k, np.float32)
    v = np.asarray(v, np.float32)
    Wq = np.asarray(Wq, np.float32)
    Wk = np.asarray(Wk, np.float32)
    Wv = np.asarray(Wv, np.float32)
    Wo = np.asarray(Wo, np.float32)
    bq = np.asarray(bq, np.float32)
    bk = np.asarray(bk, np.float32)
    bv = np.asarray(bv, np.float32)
    mask = np.asarray(attention_mask)

    in_maps = []
    for c in range(8):
        bc, hg = c // 2, c % 2
        cs = slice(hg * C, (hg + 1) * C)
        wv_aug = np.zeros((DIN, VW), np.float32)
        bv_aug = np.zeros((1, VW), np.float32)
        for h in range(NHC):
            src = slice(hg * C + h * HD, hg * C + (h + 1) * HD)
            dst = slice(h * HW1, h * HW1 + HD)
            wv_aug[:, dst] = Wv[:, src]
            bv_aug[0, dst] = bv[src]
            bv_aug[0, h * HW1 + HD] = 1.0
        in_maps.append({
            "qT": np.ascontiguousarray(q[bc].T),
            "kT": np.ascontiguousarray(k[bc].T),
            "vT": np.ascontiguousarray(v[bc].T),
            "wq": np.ascontiguousarray(Wq[:, cs]),
            "wk": np.ascontiguousarray(Wk[:, cs]),
            "wv": wv_aug,
            "wo": np.ascontiguousarray(Wo[cs, :]).astype(ml_dtypes.bfloat16),
            "bq": np.ascontiguousarray(bq[cs]),
            "bk": np.ascontiguousarray(bk[cs]),
            "bva": bv_aug,
            "onesd": np.ones((1, 512), np.float32),
            "mm": mask[bc].astype(np.float32),
        })
    return in_maps


def kernel(q, k, v, attention_mask, Wq, bq, Wk, bk, Wv, bv, Wo, bo, _trace=False):
    in_maps = make_in_maps(
        q, k, v, attention_mask, Wq, bq, Wk, bk, Wv, bv, Wo, bo
    )
    nc = _get_nc()
    import time as _time
    t0 = _time.time()
    try:
        res = run_bass_kernel_spmd(nc, in_maps, list(range(8)), trace=_trace)
    except Exception:
        if not _trace:
            raise
        res = run_bass_kernel_spmd(nc, in_maps, list(range(8)))
    kernel._last_run_seconds = _time.time() - t0
    bo = np.asarray(bo, np.float32)
    out = np.stack(
        [res.results[2 * b]["y"] + res.results[2 * b + 1]["y"] + bo
         for b in range(4)]
    ).astype(np.float32)
    if _trace:
        kernel._last_results = res
    return out

